# revision 20
# baseline (speedup 1.0000x reference)
"""GroupedQueryAttention on 8 Trainium2 NeuronCores — v5.

Problem (hardcoded): B=2, T=2048, DIM=4096, 32 q heads, 8 kv heads, hd=128.
  q = x @ Wq.T ; k,v = split(x @ Wkv.T) ; causal softmax(q k^T/sqrt(hd)) v ; out = o @ Wo.T

Sharding: hybrid data x tensor parallel over 8 cores.
  core c -> batch b = c//4, kv-head group j = c%4 (kv heads {2j,2j+1}, q heads {8j..8j+7}).

v5 (v2 1.063 ms, v4 1.165 ms):
  The v4 trace showed per-head 9-17 us tensor stalls: each in-flight AllGather
  starves the completion of concurrent small DMAs (every dma splits across all
  16 SDMA engines; the slices sharing engines with the collective crawl).  So
  phases A/B are made DMA-free in the steady state:
   - xT and Wkv resident in SBUF (strip loads only at kernel start),
   - each head's Q weights arrive as ONE contiguous 1 MB DMA (from a
     host-side pre-tiled layout), prefetched one head ahead, always issued
     while no AllGather is in flight.
  B1 (Q-proj) accumulates into ps_sT-pool tiles, freeing two PSUM banks so
  oT/den pools go back to bufs=2.  The softmax reciprocal runs on ScalarE as
  Exp(-Ln(den)) with a manually pre-placed combined exp+ln ACT-table load
  (without it walrus thrashes table sets, 65 x 2.7 us).  AV matmuls lag the
  score matmuls by two pairs; per-(h,tg) tails are emitted one group late.
"""

import sys

sys.path.insert(0, "/opt/trn_rl_repo")

import math

import numpy as np

import concourse.bass as bass
import concourse.bacc as bacc
import concourse.tile as tile
from concourse import mybir
from concourse.bass_utils import run_bass_kernel_spmd

B, T, DIM = 2, 2048, 4096
N_HEADS, N_KV, HD = 32, 8, 128
R = N_HEADS // N_KV  # 4
NCORES = 8
GROUPS = [[0, 1, 2, 3], [4, 5, 6, 7]]

HPC = 8  # q heads per core
KVPC = 2  # kv heads per core
EQ = HPC * HD  # 1024 q-proj out features per core
EKV = KVPC * HD  # 256 k (and v) out features per core
NT = T // 512  # 4 t-groups of 512
NC = DIM // 128  # 32 contraction tiles
NKB = T // 128  # 16 k-tiles per head

BF = mybir.dt.bfloat16
F32 = mybir.dt.float32
INV_SQRT_HD = 1.0 / math.sqrt(HD)


def _exp_ln_set_id(arch):
    try:
        from concourse.hw_specs import get_activation_tables

        tables = get_activation_tables(arch)
    except Exception:
        return None
    want = {mybir.ActivationFunctionType.Exp, mybir.ActivationFunctionType.Ln}
    for idx, (name, funcs) in enumerate(tables.items()):
        if want <= funcs:
            return idx
    return None


def build():
    nc = bacc.Bacc("TRN2", num_devices=NCORES)

    # ---- external I/O (per-core data differs, program is SPMD-identical) ----
    xT = nc.dram_tensor("xT", [DIM, T], BF, kind="ExternalInput")  # x[b].T
    wkvT = nc.dram_tensor("wkvT", [DIM, 4 * HD], BF, kind="ExternalInput")
    # Q weights pre-tiled into SBUF layout: row h*128+p, col cb*128+q holds
    # WqT[cb*128+p, h*128+q]; head h loads as ONE contiguous [128,4096] DMA.
    wqsb = nc.dram_tensor("wqsb", [HPC * 128, NC * 128], BF,
                          kind="ExternalInput")
    woT = nc.dram_tensor("woT", [DIM, EQ], BF, kind="ExternalInput")
    maskA = nc.dram_tensor("maskA", [128, 1024], BF, kind="ExternalInput")
    maskB = nc.dram_tensor("maskB", [128, 1024], BF, kind="ExternalInput")
    ident = nc.dram_tensor("ident", [128, 128], BF, kind="ExternalInput")
    ones_in = nc.dram_tensor("ones_in", [128, 1], BF, kind="ExternalInput")
    out_part = nc.dram_tensor("out_part", [EQ, T], F32, kind="ExternalOutput")

    with tile.TileContext(nc) as tc:
        # combined exp+ln table load, placed first on the scalar queue so the
        # table-load fixpoint pass sees both functions covered on every path
        set_id = _exp_ln_set_id(nc.m.arch)
        if set_id is not None:
            nc.scalar.add_instruction(
                mybir.InstLoadActFuncSet(
                    name=nc.get_next_instruction_name(),
                    act_func_set_id=set_id,
                    ins=[],
                    outs=[],
                )
            )
        with (
            tc.tile_pool(name="persist", bufs=1) as persist,
            tc.tile_pool(name="work", bufs=3) as work,
            tc.tile_pool(name="dram2", bufs=1, space="DRAM") as dram2,
        ):
            # ---------------- constants ----------------
            maskA_sb = persist.tile([128, 1024], BF)
            nc.sync.dma_start(out=maskA_sb[:], in_=maskA[:, :])
            maskB_sb = persist.tile([128, 1024], BF)
            nc.sync.dma_start(out=maskB_sb[:], in_=maskB[:, :])
            ident_sb = persist.tile([128, 128], BF)
            nc.sync.dma_start(out=ident_sb[:], in_=ident[:, :])
            ones_sb = persist.tile([128, 1], BF)
            nc.sync.dma_start(out=ones_sb[:], in_=ones_in[:, :])

            kt_sb = persist.tile([128, KVPC * T], BF)  # KT per kv head
            v_sb = persist.tile([128, KVPC * T], BF)  # V[t,dv] tiles

            og_in = []
            og_out = []
            for h in range(HPC - 1):
                og_in.append(dram2.tile([128, T], BF, name=f"og_in_{h}"))
                og_out.append(dram2.tile([4 * 128, T], BF, name=f"og_out_{h}"))
            og_in7 = [dram2.tile([128, 1024], BF, name=f"og_in7_{i}",
                                 tag=f"og_in7_{i}") for i in range(2)]
            og_out7 = [dram2.tile([4 * 128, 1024], BF, name=f"og_out7_{i}",
                                  tag=f"og_out7_{i}") for i in range(2)]

            with tc.tile_pool(name="poolx", bufs=1) as poolx:
                xT_sb = poolx.tile([128, NC * T], BF)  # strip cb at [cb*T,..)

                def load_wq(h):
                    t = work.tile([128, NC * 128], BF, tag="wqh", bufs=2,
                                  name=f"wqh{h}")
                    nc.sync.dma_start(
                        out=t[:], in_=wqsb[h * 128:(h + 1) * 128, :],
                    )
                    return t

                # ------------- phase A: K/V projections -------------
                with (
                    tc.tile_pool(name="poolA", bufs=1) as poolA,
                    tc.tile_pool(name="psum_a", bufs=1, space="PSUM") as psum_a,
                ):
                    wkv_sb = poolA.tile([128, NC * 512], BF)
                    vt_sb = poolA.tile([128, KVPC * T], BF)
                    # interleave xT / wkv strip loads so neither ring
                    # head-blocks the first matmuls
                    for cb in range(NC):
                        nc.sync.dma_start(
                            out=xT_sb[:, cb * T:(cb + 1) * T],
                            in_=xT[cb * 128:(cb + 1) * 128, :],
                        )
                        nc.sync.dma_start(
                            out=wkv_sb[:, cb * 512:(cb + 1) * 512],
                            in_=wkvT[cb * 128:(cb + 1) * 128, :],
                        )
                    wqh_next = load_wq(0)  # head 0 Q weights prefetch

                    for tgp in range(2):
                        accs = [[psum_a.tile([128, 512], F32, tag=f"acc{i}{j}",
                                             name=f"acc{i}{j}")
                                 for j in range(2)] for i in range(4)]
                        for cb in range(NC):
                            for i in range(4):
                                w = wkv_sb[:, cb * 512 + i * 128:
                                           cb * 512 + (i + 1) * 128]
                                for j in range(2):
                                    t0 = (2 * tgp + j) * 512
                                    nc.tensor.matmul(
                                        accs[i][j][:], w,
                                        xT_sb[:, cb * T + t0:cb * T + t0 + 512],
                                        start=(cb == 0), stop=(cb == NC - 1),
                                    )
                        for i in range(4):
                            dst = (kt_sb if i < 2 else vt_sb)
                            g = i % 2
                            for j in range(2):
                                t0 = (2 * tgp + j) * 512
                                nc.vector.tensor_copy(
                                    dst[:, g * T + t0:g * T + t0 + 512],
                                    accs[i][j][:],
                                )
                    # V = VT.T per 128x128 tile (PE transpose-mode)
                    for g in range(KVPC):
                        for kb in range(NKB):
                            tp = psum_a.tile([128, 128], BF,
                                             tag=f"acc{kb % 4}{g}")
                            nc.tensor.transpose(
                                tp[:],
                                vt_sb[:, g * T + kb * 128:g * T + (kb + 1) * 128],
                                ident_sb[:],
                            )
                            nc.vector.tensor_copy(
                                v_sb[:, (g * NKB + kb) * 128:
                                     (g * NKB + kb + 1) * 128],
                                tp[:],
                            )

                # ------------- phase B: per-head Q-proj + attention ----------
                with (
                    tc.tile_pool(name="ps_sT", bufs=2, space="PSUM") as ps_sT,
                    tc.tile_pool(name="ps_oT", bufs=2, space="PSUM") as ps_oT,
                    tc.tile_pool(name="ps_den", bufs=2, space="PSUM") as ps_den,
                    tc.tile_pool(name="work2", bufs=3) as work2,
                ):
                    pending_tail = [None]

                    def flush_tail():
                        if pending_tail[0] is not None:
                            pending_tail[0]()
                            pending_tail[0] = None

                    for h in range(HPC):
                        g = h // R  # local kv head
                        kt_g = kt_sb[:, g * T:(g + 1) * T]
                        wqh_sb = wqh_next

                        # ----- B1: project Q_h into qh_sb [128, T] -----
                        qh_sb = work2.tile([128, T], BF, tag="qh", bufs=2)
                        for tgp in range(2):
                            qacc = ps_sT.tile([128, 1024], F32, tag="sT2",
                                              name="qacc")
                            for cb in range(NC):
                                w = wqh_sb[:, cb * 128:(cb + 1) * 128]
                                for j in range(2):
                                    t0 = (2 * tgp + j) * 512
                                    nc.tensor.matmul(
                                        qacc[:, j * 512:(j + 1) * 512], w,
                                        xT_sb[:, cb * T + t0:cb * T + t0 + 512],
                                        start=(cb == 0), stop=(cb == NC - 1),
                                        skip_group_check=True,
                                    )
                                if tgp == 0 and cb == 2:
                                    flush_tail()  # previous head's tg3 tail
                            for j in range(2):
                                t0 = (2 * tgp + j) * 512
                                nc.vector.tensor_copy(
                                    qh_sb[:, t0:t0 + 512],
                                    qacc[:, j * 512:(j + 1) * 512],
                                )

                        # ----- B2: attention for head h -----
                        oTh = work2.tile([128, T], BF, tag="oTh", bufs=2)
                        for tg in range(NT):
                            npairs = 2 * tg + 2
                            qs = qh_sb[:, tg * 512:(tg + 1) * 512]
                            oT_acc = ps_oT.tile([128, 512], F32, tag="oT")
                            expsum = work2.tile([128, 1024], BF, tag="expsum",
                                                bufs=2)
                            expps = []

                            if h + 1 < HPC and tg == 0:
                                # next head's Q weights: issued while no
                                # AllGather is in flight
                                wqh_next = load_wq(h + 1)

                            def emit_av(p, expp, _oT=oT_acc, _np=npairs, _g=g):
                                first = (p == 0)
                                last = (p == _np - 1)
                                for j in range(2):
                                    kb = 2 * p + j
                                    nc.tensor.matmul(
                                        _oT[:],
                                        v_sb[:, (_g * NKB + kb) * 128:
                                             (_g * NKB + kb + 1) * 128],
                                        expp[:, j * 512:(j + 1) * 512],
                                        start=(first and j == 0),
                                        stop=(last and j == 1),
                                        skip_group_check=True,
                                    )

                            for p in range(npairs):
                                sT2 = ps_sT.tile([128, 1024], F32, tag="sT2")
                                for j in range(2):
                                    kb = 2 * p + j
                                    nc.tensor.matmul(
                                        sT2[:, j * 512:(j + 1) * 512],
                                        kt_g[:, kb * 128:(kb + 1) * 128],
                                        qs,
                                        start=True, stop=True,
                                        skip_group_check=True,
                                    )
                                if p == 1:
                                    flush_tail()
                                expp = work2.tile([128, 1024], BF, tag="expT2",
                                                  bufs=3)
                                nc.scalar.activation(
                                    expp[:], sT2[:],
                                    mybir.ActivationFunctionType.Exp,
                                    scale=INV_SQRT_HD,
                                )
                                if p == npairs - 2:
                                    nc.vector.tensor_tensor(
                                        expp[:], expp[:], maskA_sb[:],
                                        mybir.AluOpType.mult,
                                    )
                                elif p == npairs - 1:
                                    nc.vector.tensor_tensor(
                                        expp[:], expp[:], maskB_sb[:],
                                        mybir.AluOpType.mult,
                                    )
                                if p == 0:
                                    nc.vector.tensor_copy(expsum[:], expp[:])
                                else:
                                    nc.vector.tensor_tensor(
                                        expsum[:], expsum[:], expp[:],
                                        mybir.AluOpType.add,
                                    )
                                expps.append(expp)
                                if p >= 2:
                                    emit_av(p - 2, expps[p - 2])
                            for p in (npairs - 2, npairs - 1):
                                emit_av(p, expps[p])

                            def tail(_h=h, _tg=tg, _oT=oT_acc, _es=expsum,
                                     _oTh=oTh):
                                den_acc = ps_den.tile([1, 512], F32, tag="den")
                                for j in range(2):
                                    nc.tensor.matmul(
                                        den_acc[:], ones_sb[:],
                                        _es[:, j * 512:(j + 1) * 512],
                                        start=(j == 0), stop=(j == 1),
                                        skip_group_check=True,
                                    )
                                # 1/den = Exp(-Ln(den)) on ScalarE (combined
                                # table set pre-loaded; VectorE's iterative
                                # divide is 3.3us and convoys its FIFO)
                                lnden = work2.tile([1, 512], F32, tag="lnden")
                                nc.scalar.activation(
                                    lnden[:], den_acc[:],
                                    mybir.ActivationFunctionType.Ln,
                                )
                                recip = work2.tile([1, 512], F32, tag="recip")
                                nc.scalar.activation(
                                    recip[:], lnden[:],
                                    mybir.ActivationFunctionType.Exp,
                                    scale=-1.0,
                                )
                                recip_b = work2.tile([128, 512], F32,
                                                     tag="recip_b", bufs=2)
                                nc.gpsimd.partition_broadcast(recip_b[:],
                                                              recip[:])
                                nc.vector.tensor_tensor(
                                    _oTh[:, _tg * 512:(_tg + 1) * 512],
                                    _oT[:],
                                    recip_b[:],
                                    mybir.AluOpType.mult,
                                )
                                # ship completed halves/heads
                                if _h == HPC - 1 and _tg in (1, 3):
                                    i7 = _tg // 2
                                    nc.sync.dma_start(
                                        out=og_in7[i7][:],
                                        in_=_oTh[:, i7 * 1024:(i7 + 1) * 1024],
                                    )
                                    nc.gpsimd.collective_compute(
                                        "AllGather",
                                        mybir.AluOpType.bypass,
                                        replica_groups=GROUPS,
                                        ins=[og_in7[i7].opt()],
                                        outs=[og_out7[i7].opt()],
                                    )
                                elif _h < HPC - 1 and _tg == 3:
                                    nc.sync.dma_start(
                                        out=og_in[_h][:], in_=_oTh[:, :],
                                    )
                                    nc.gpsimd.collective_compute(
                                        "AllGather",
                                        mybir.AluOpType.bypass,
                                        replica_groups=GROUPS,
                                        ins=[og_in[_h].opt()],
                                        outs=[og_out[_h].opt()],
                                    )

                            flush_tail()
                            pending_tail[0] = tail
                    flush_tail()  # last head's tg3 tail

            # poolx released: xT's 16.8 MB of SBUF freed for phase C weights.
            with (
                tc.tile_pool(name="p3pool", bufs=1) as p3pool,
                tc.tile_pool(name="work3", bufs=3) as work3,
            ):
                woT_sb = p3pool.tile([128, NC * EQ], BF)
                for cb in range(NC):
                    nc.sync.dma_start(
                        out=woT_sb[:, cb * EQ:(cb + 1) * EQ],
                        in_=woT[cb * 128:(cb + 1) * 128, :],
                    )

                # ------------- phase C: outT slice = WoT.T @ oT_full ---------
                eb_order = [rr * HPC + hh for hh in range(HPC) for rr in range(4)]
                with tc.tile_pool(name="ps_out", bufs=1, space="PSUM") as ps_out:
                    for ocp in range(2):
                        for tgp in range(2):
                            accs = [[ps_out.tile([128, 512], F32,
                                                 tag=f"out{oi}{j}",
                                                 name=f"out{oi}{j}")
                                     for j in range(2)] for oi in range(4)]
                            for ei, eb in enumerate(eb_order):
                                r, hl = eb // HPC, eb % HPC
                                rhs_t = work3.tile([128, 1024], BF, tag="rhs",
                                                   bufs=12)
                                if hl == HPC - 1:
                                    src = og_out7[tgp][r * 128:(r + 1) * 128, :]
                                else:
                                    src = og_out[hl][r * 128:(r + 1) * 128,
                                                     tgp * 1024:(tgp + 1) * 1024]
                                nc.sync.dma_start(out=rhs_t[:], in_=src)
                                for oi in range(4):
                                    oc = ocp * 4 + oi
                                    w = woT_sb[:, eb * EQ + oc * 128:
                                               eb * EQ + (oc + 1) * 128]
                                    for j in range(2):
                                        nc.tensor.matmul(
                                            accs[oi][j][:],
                                            w,
                                            rhs_t[:, j * 512:(j + 1) * 512],
                                            start=(ei == 0),
                                            stop=(ei == NC - 1),
                                        )
                            for oi in range(4):
                                oc = ocp * 4 + oi
                                for j in range(2):
                                    t0 = tgp * 1024 + j * 512
                                    ev = work3.tile([128, 512], F32, tag="ev",
                                                    bufs=4)
                                    nc.vector.tensor_copy(ev[:], accs[oi][j][:])
                                    nc.sync.dma_start(
                                        out=out_part[oc * 128:(oc + 1) * 128,
                                                     t0:t0 + 512],
                                        in_=ev[:],
                                    )
    nc.finalize()
    return nc


_NC_CACHE = None


def _get_nc():
    global _NC_CACHE
    if _NC_CACHE is None:
        _NC_CACHE = build()
    return _NC_CACHE


def kernel(x, Wq, Wkv, Wo):
    x = np.asarray(x, dtype=np.float32)
    Wq = np.asarray(Wq, dtype=np.float32)
    Wkv = np.asarray(Wkv, dtype=np.float32)
    Wo = np.asarray(Wo, dtype=np.float32)

    try:
        import ml_dtypes

        bf16 = ml_dtypes.bfloat16
    except ImportError:  # pragma: no cover
        import jax.numpy as jnp

        bf16 = jnp.bfloat16

    xT_b = [np.ascontiguousarray(x[b].T).astype(bf16) for b in range(B)]

    kl = np.arange(128)[:, None]
    ql = np.arange(512)[None, :]
    masks = []
    for pos in range(2):
        cols = []
        for j in range(2):
            jd = 2 * pos + j
            cols.append((kl <= ql - 128 * jd).astype(np.float32))
        masks.append(np.concatenate(cols, axis=1).astype(bf16))
    maskA_np, maskB_np = masks

    ident = np.eye(128, dtype=np.float32).astype(bf16)
    ones = np.ones((128, 1), dtype=np.float32).astype(bf16)

    in_maps = []
    for c in range(NCORES):
        b, j = c // 4, c % 4
        wq_l = Wq[EQ * j:EQ * (j + 1), :]  # [1024, 4096]
        wk_l = Wkv[EKV * j:EKV * (j + 1), :]  # [256, 4096]
        wv_l = Wkv[N_KV * HD + EKV * j:N_KV * HD + EKV * (j + 1), :]
        wkv = np.concatenate([wk_l, wv_l], axis=0)  # [512, 4096]
        wkvT_l = np.ascontiguousarray(wkv.T).astype(bf16)  # [4096, 512]
        # Q weights in SBUF layout: row h*128+p, col cb*128+q =
        # WqT[cb*128+p, h*128+q]
        wqT_l = wq_l.T.reshape(NC, 128, HPC, 128)  # (cb, p, h, q)
        wqsb_l = np.ascontiguousarray(
            wqT_l.transpose(2, 1, 0, 3).reshape(HPC * 128, NC * 128)
        ).astype(bf16)
        woT_l = np.ascontiguousarray(Wo[EQ * j:EQ * (j + 1), :].T).astype(bf16)
        in_maps.append(
            {
                "xT": xT_b[b],
                "wkvT": wkvT_l,
                "wqsb": wqsb_l,
                "woT": woT_l,
                "maskA": maskA_np,
                "maskB": maskB_np,
                "ident": ident,
                "ones_in": ones,
            }
        )

    nc = _get_nc()
    res = run_bass_kernel_spmd(nc, in_maps, core_ids=list(range(NCORES)))

    out = np.empty((B, T, DIM), dtype=np.float32)
    for b in range(B):
        outT = np.concatenate(
            [res.results[b * 4 + j]["out_part"] for j in range(4)], axis=0
        )  # [4096, 2048]
        out[b] = outT.T
    return out


# revision 28
# speedup vs baseline: 1.0228x; 1.0228x over previous
"""GroupedQueryAttention on 8 Trainium2 NeuronCores — v5.

Problem (hardcoded): B=2, T=2048, DIM=4096, 32 q heads, 8 kv heads, hd=128.
  q = x @ Wq.T ; k,v = split(x @ Wkv.T) ; causal softmax(q k^T/sqrt(hd)) v ; out = o @ Wo.T

Sharding: hybrid data x tensor parallel over 8 cores.
  core c -> batch b = c//4, kv-head group j = c%4 (kv heads {2j,2j+1}, q heads {8j..8j+7}).

v5 (v2 1.063 ms, v4 1.165 ms):
  The v4 trace showed per-head 9-17 us tensor stalls: each in-flight AllGather
  starves the completion of concurrent small DMAs (every dma splits across all
  16 SDMA engines; the slices sharing engines with the collective crawl).  So
  phases A/B are made DMA-free in the steady state:
   - xT and Wkv resident in SBUF (strip loads only at kernel start),
   - each head's Q weights arrive as ONE contiguous 1 MB DMA (from a
     host-side pre-tiled layout), prefetched one head ahead, always issued
     while no AllGather is in flight.
  B1 (Q-proj) accumulates into ps_sT-pool tiles, freeing two PSUM banks so
  oT/den pools go back to bufs=2.  The softmax reciprocal runs on ScalarE as
  Exp(-Ln(den)) with a manually pre-placed combined exp+ln ACT-table load
  (without it walrus thrashes table sets, 65 x 2.7 us).  AV matmuls lag the
  score matmuls by two pairs; per-(h,tg) tails are emitted one group late.
"""

import sys

sys.path.insert(0, "/opt/trn_rl_repo")

import math

import numpy as np

import concourse.bass as bass
import concourse.bacc as bacc
import concourse.tile as tile
from concourse import mybir
from concourse.bass_utils import run_bass_kernel_spmd

B, T, DIM = 2, 2048, 4096
N_HEADS, N_KV, HD = 32, 8, 128
R = N_HEADS // N_KV  # 4
NCORES = 8
GROUPS = [[0, 1, 2, 3], [4, 5, 6, 7]]

HPC = 8  # q heads per core
KVPC = 2  # kv heads per core
EQ = HPC * HD  # 1024 q-proj out features per core
EKV = KVPC * HD  # 256 k (and v) out features per core
NT = T // 512  # 4 t-groups of 512
NC = DIM // 128  # 32 contraction tiles
NKB = T // 128  # 16 k-tiles per head

BF = mybir.dt.bfloat16
F32 = mybir.dt.float32
INV_SQRT_HD = 1.0 / math.sqrt(HD)


def _exp_ln_set_id(arch):
    try:
        from concourse.hw_specs import get_activation_tables

        tables = get_activation_tables(arch)
    except Exception:
        return None
    want = {mybir.ActivationFunctionType.Exp, mybir.ActivationFunctionType.Ln}
    for idx, (name, funcs) in enumerate(tables.items()):
        if want <= funcs:
            return idx
    return None


def build():
    nc = bacc.Bacc("TRN2", num_devices=NCORES)

    # ---- external I/O (per-core data differs, program is SPMD-identical) ----
    xT = nc.dram_tensor("xT", [DIM, T], BF, kind="ExternalInput")  # x[b].T
    wkvT = nc.dram_tensor("wkvT", [DIM, 4 * HD], BF, kind="ExternalInput")
    # Q weights pre-tiled into SBUF layout: row h*128+p, col cb*128+q holds
    # WqT[cb*128+p, h*128+q]; head h loads as ONE contiguous [128,4096] DMA.
    wqsb = nc.dram_tensor("wqsb", [HPC * 128, NC * 128], BF,
                          kind="ExternalInput")
    woT = nc.dram_tensor("woT", [DIM, EQ], BF, kind="ExternalInput")
    maskA = nc.dram_tensor("maskA", [128, 1024], BF, kind="ExternalInput")
    maskB = nc.dram_tensor("maskB", [128, 1024], BF, kind="ExternalInput")
    ident = nc.dram_tensor("ident", [128, 128], BF, kind="ExternalInput")
    ones_in = nc.dram_tensor("ones_in", [128, 1], BF, kind="ExternalInput")
    out_part = nc.dram_tensor("out_part", [EQ, T], F32, kind="ExternalOutput")

    with tile.TileContext(nc) as tc:
        # combined exp+ln table load, placed first on the scalar queue so the
        # table-load fixpoint pass sees both functions covered on every path
        set_id = _exp_ln_set_id(nc.m.arch)
        if set_id is not None:
            nc.scalar.add_instruction(
                mybir.InstLoadActFuncSet(
                    name=nc.get_next_instruction_name(),
                    act_func_set_id=set_id,
                    ins=[],
                    outs=[],
                )
            )
        with (
            tc.tile_pool(name="persist", bufs=1) as persist,
            tc.tile_pool(name="work", bufs=3) as work,
            tc.tile_pool(name="dram2", bufs=1, space="DRAM") as dram2,
        ):
            # ---------------- constants ----------------
            # (DMAs for these are emitted after the first data strips so the
            # first matmuls aren't queued behind them)
            maskA_sb = persist.tile([128, 1024], BF)
            maskB_sb = persist.tile([128, 1024], BF)
            ident_sb = persist.tile([128, 128], BF)
            ones_sb = persist.tile([128, 1], BF)

            def load_consts():
                nc.sync.dma_start(out=maskA_sb[:], in_=maskA[:, :])
                nc.sync.dma_start(out=maskB_sb[:], in_=maskB[:, :])
                nc.sync.dma_start(out=ident_sb[:], in_=ident[:, :])
                nc.sync.dma_start(out=ones_sb[:], in_=ones_in[:, :])

            kt_sb = persist.tile([128, KVPC * T], BF)  # KT per kv head
            v_sb = persist.tile([128, KVPC * T], BF)  # V[t,dv] tiles

            og_in = []
            og_out = []
            for h in range(HPC - 1):
                og_in.append(dram2.tile([128, T], BF, name=f"og_in_{h}"))
                og_out.append(dram2.tile([4 * 128, T], BF, name=f"og_out_{h}"))
            og_in7 = [dram2.tile([128, 1024], BF, name=f"og_in7_{i}",
                                 tag=f"og_in7_{i}") for i in range(2)]
            og_out7 = [dram2.tile([4 * 128, 1024], BF, name=f"og_out7_{i}",
                                  tag=f"og_out7_{i}") for i in range(2)]

            with tc.tile_pool(name="poolx", bufs=1) as poolx:
                xT_sb = poolx.tile([128, NC * T], BF)  # strip cb at [cb*T,..)

                def load_wq(h):
                    t = work.tile([128, NC * 128], BF, tag="wqh", bufs=2,
                                  name=f"wqh{h}")
                    nc.sync.dma_start(
                        out=t[:], in_=wqsb[h * 128:(h + 1) * 128, :],
                    )
                    return t

                # ------------- phase A: K/V projections -------------
                with (
                    tc.tile_pool(name="poolA", bufs=1) as poolA,
                    tc.tile_pool(name="psum_a", bufs=1, space="PSUM") as psum_a,
                ):
                    wkv_sb = poolA.tile([128, NC * 512], BF)
                    vt_sb = poolA.tile([128, KVPC * T], BF)
                    # interleave xT / wkv strip loads so neither ring
                    # head-blocks the first matmuls; pass 0 reads t < 1024
                    # only, so ship those halves first for a fast ramp
                    for cb in range(NC):
                        nc.sync.dma_start(
                            out=xT_sb[:, cb * T:cb * T + 1024],
                            in_=xT[cb * 128:(cb + 1) * 128, 0:1024],
                        )
                        nc.sync.dma_start(
                            out=wkv_sb[:, cb * 512:(cb + 1) * 512],
                            in_=wkvT[cb * 128:(cb + 1) * 128, :],
                        )
                        if cb == 3:
                            load_consts()
                    for cb in range(NC):
                        nc.sync.dma_start(
                            out=xT_sb[:, cb * T + 1024:(cb + 1) * T],
                            in_=xT[cb * 128:(cb + 1) * 128, 1024:T],
                        )
                    wqh_next = load_wq(0)  # head 0 Q weights prefetch

                    for tgp in range(2):
                        accs = [[psum_a.tile([128, 512], F32, tag=f"acc{i}{j}",
                                             name=f"acc{i}{j}")
                                 for j in range(2)] for i in range(4)]
                        for cb in range(NC):
                            for i in range(4):
                                w = wkv_sb[:, cb * 512 + i * 128:
                                           cb * 512 + (i + 1) * 128]
                                for j in range(2):
                                    t0 = (2 * tgp + j) * 512
                                    nc.tensor.matmul(
                                        accs[i][j][:], w,
                                        xT_sb[:, cb * T + t0:cb * T + t0 + 512],
                                        start=(cb == 0), stop=(cb == NC - 1),
                                    )
                        for i in range(4):
                            dst = (kt_sb if i < 2 else vt_sb)
                            g = i % 2
                            for j in range(2):
                                t0 = (2 * tgp + j) * 512
                                nc.vector.tensor_copy(
                                    dst[:, g * T + t0:g * T + t0 + 512],
                                    accs[i][j][:],
                                )
                    # V = VT.T per 128x128 tile (PE transpose-mode)
                    for g in range(KVPC):
                        for kb in range(NKB):
                            tp = psum_a.tile([128, 128], BF,
                                             tag=f"acc{kb % 4}{g}")
                            nc.tensor.transpose(
                                tp[:],
                                vt_sb[:, g * T + kb * 128:g * T + (kb + 1) * 128],
                                ident_sb[:],
                            )
                            nc.vector.tensor_copy(
                                v_sb[:, (g * NKB + kb) * 128:
                                     (g * NKB + kb + 1) * 128],
                                tp[:],
                            )

                # ------------- phase B: per-head Q-proj + attention ----------
                with (
                    tc.tile_pool(name="ps_sT", bufs=2, space="PSUM") as ps_sT,
                    tc.tile_pool(name="ps_oT", bufs=2, space="PSUM") as ps_oT,
                    tc.tile_pool(name="ps_den", bufs=2, space="PSUM") as ps_den,
                    tc.tile_pool(name="work2", bufs=3) as work2,
                ):
                    pending_tail = [None]

                    def flush_tail():
                        if pending_tail[0] is not None:
                            pending_tail[0]()
                            pending_tail[0] = None

                    for h in range(HPC):
                        g = h // R  # local kv head
                        kt_g = kt_sb[:, g * T:(g + 1) * T]
                        wqh_sb = wqh_next

                        # ----- B1: project Q_h -----
                        # separate lo/hi Q tiles per t-half: attention tg 0/1
                        # then only waits the pass-0 drain, not all 4 copies
                        qh_halves = [
                            work2.tile([128, 1024], BF, tag="qh_lo", bufs=2,
                                       name="qh_lo"),
                            work2.tile([128, 1024], BF, tag="qh_hi", bufs=2,
                                       name="qh_hi"),
                        ]
                        for tgp in range(2):
                            qacc = ps_sT.tile([128, 1024], F32, tag="sT2",
                                              name="qacc")
                            for cb in range(NC):
                                w = wqh_sb[:, cb * 128:(cb + 1) * 128]
                                for j in range(2):
                                    t0 = (2 * tgp + j) * 512
                                    nc.tensor.matmul(
                                        qacc[:, j * 512:(j + 1) * 512], w,
                                        xT_sb[:, cb * T + t0:cb * T + t0 + 512],
                                        start=(cb == 0), stop=(cb == NC - 1),
                                        skip_group_check=True,
                                    )
                                if tgp == 0 and cb == 2:
                                    flush_tail()  # previous head's tg3 tail
                            for j in range(2):
                                nc.vector.tensor_copy(
                                    qh_halves[tgp][:, j * 512:(j + 1) * 512],
                                    qacc[:, j * 512:(j + 1) * 512],
                                )

                        # ----- B2: attention for head h -----
                        oTh = work2.tile([128, T], BF, tag="oTh", bufs=2)
                        for tg in range(NT):
                            npairs = 2 * tg + 2
                            qs = qh_halves[tg // 2][:, (tg % 2) * 512:
                                                    (tg % 2 + 1) * 512]
                            oT_acc = ps_oT.tile([128, 512], F32, tag="oT")
                            expsum = work2.tile([128, 1024], BF, tag="expsum",
                                                bufs=2)
                            expps = []

                            if h + 1 < HPC and tg == 0:
                                # next head's Q weights: issued while no
                                # AllGather is in flight
                                wqh_next = load_wq(h + 1)

                            def emit_av(p, expp, _oT=oT_acc, _np=npairs, _g=g):
                                first = (p == 0)
                                last = (p == _np - 1)
                                for j in range(2):
                                    kb = 2 * p + j
                                    nc.tensor.matmul(
                                        _oT[:],
                                        v_sb[:, (_g * NKB + kb) * 128:
                                             (_g * NKB + kb + 1) * 128],
                                        expp[:, j * 512:(j + 1) * 512],
                                        start=(first and j == 0),
                                        stop=(last and j == 1),
                                        skip_group_check=True,
                                    )

                            for p in range(npairs):
                                sT2 = ps_sT.tile([128, 1024], F32, tag="sT2")
                                for j in range(2):
                                    kb = 2 * p + j
                                    nc.tensor.matmul(
                                        sT2[:, j * 512:(j + 1) * 512],
                                        kt_g[:, kb * 128:(kb + 1) * 128],
                                        qs,
                                        start=True, stop=True,
                                        skip_group_check=True,
                                    )
                                if p == 1:
                                    flush_tail()
                                expp = work2.tile([128, 1024], BF, tag="expT2",
                                                  bufs=3)
                                nc.scalar.activation(
                                    expp[:], sT2[:],
                                    mybir.ActivationFunctionType.Exp,
                                    scale=INV_SQRT_HD,
                                )
                                if p == npairs - 2:
                                    nc.vector.tensor_tensor(
                                        expp[:], expp[:], maskA_sb[:],
                                        mybir.AluOpType.mult,
                                    )
                                elif p == npairs - 1:
                                    nc.vector.tensor_tensor(
                                        expp[:], expp[:], maskB_sb[:],
                                        mybir.AluOpType.mult,
                                    )
                                if p == 0:
                                    nc.vector.tensor_copy(expsum[:], expp[:])
                                else:
                                    nc.vector.tensor_tensor(
                                        expsum[:], expsum[:], expp[:],
                                        mybir.AluOpType.add,
                                    )
                                expps.append(expp)
                                if p >= 2:
                                    emit_av(p - 2, expps[p - 2])
                            for p in (npairs - 2, npairs - 1):
                                emit_av(p, expps[p])

                            def tail(_h=h, _tg=tg, _oT=oT_acc, _es=expsum,
                                     _oTh=oTh):
                                den_acc = ps_den.tile([1, 512], F32, tag="den")
                                for j in range(2):
                                    nc.tensor.matmul(
                                        den_acc[:], ones_sb[:],
                                        _es[:, j * 512:(j + 1) * 512],
                                        start=(j == 0), stop=(j == 1),
                                        skip_group_check=True,
                                    )
                                # 1/den = Exp(-Ln(den)) on ScalarE (combined
                                # table set pre-loaded; VectorE's iterative
                                # divide is 3.3us and convoys its FIFO)
                                lnden = work2.tile([1, 512], F32, tag="lnden")
                                nc.scalar.activation(
                                    lnden[:], den_acc[:],
                                    mybir.ActivationFunctionType.Ln,
                                )
                                recip = work2.tile([1, 512], F32, tag="recip")
                                nc.scalar.activation(
                                    recip[:], lnden[:],
                                    mybir.ActivationFunctionType.Exp,
                                    scale=-1.0,
                                )
                                recip_b = work2.tile([128, 512], F32,
                                                     tag="recip_b", bufs=2)
                                nc.gpsimd.partition_broadcast(recip_b[:],
                                                              recip[:])
                                nc.vector.tensor_tensor(
                                    _oTh[:, _tg * 512:(_tg + 1) * 512],
                                    _oT[:],
                                    recip_b[:],
                                    mybir.AluOpType.mult,
                                )
                                # ship completed halves/heads
                                if _h == HPC - 1 and _tg in (1, 3):
                                    i7 = _tg // 2
                                    nc.sync.dma_start(
                                        out=og_in7[i7][:],
                                        in_=_oTh[:, i7 * 1024:(i7 + 1) * 1024],
                                    )
                                    nc.gpsimd.collective_compute(
                                        "AllGather",
                                        mybir.AluOpType.bypass,
                                        replica_groups=GROUPS,
                                        ins=[og_in7[i7].opt()],
                                        outs=[og_out7[i7].opt()],
                                    )
                                elif _h < HPC - 1 and _tg == 3:
                                    nc.sync.dma_start(
                                        out=og_in[_h][:], in_=_oTh[:, :],
                                    )
                                    nc.gpsimd.collective_compute(
                                        "AllGather",
                                        mybir.AluOpType.bypass,
                                        replica_groups=GROUPS,
                                        ins=[og_in[_h].opt()],
                                        outs=[og_out[_h].opt()],
                                    )

                            flush_tail()
                            pending_tail[0] = tail
                    flush_tail()  # last head's tg3 tail

            # poolx released: xT's 16.8 MB of SBUF freed for phase C weights.
            with (
                tc.tile_pool(name="p3pool", bufs=1) as p3pool,
                tc.tile_pool(name="work3", bufs=3) as work3,
            ):
                woT_sb = p3pool.tile([128, NC * EQ], BF)

                def load_wo(cb):
                    nc.sync.dma_start(
                        out=woT_sb[:, cb * EQ:(cb + 1) * EQ],
                        in_=woT[cb * 128:(cb + 1) * 128, :],
                    )



                # ------------- phase C: outT slice = WoT.T @ oT_full ---------
                eb_order = [rr * HPC + hh for hh in range(HPC) for rr in range(4)]
                with tc.tile_pool(name="ps_out", bufs=1, space="PSUM") as ps_out:
                    for ocp in range(2):
                        for tgp in range(2):
                            accs = [[ps_out.tile([128, 512], F32,
                                                 tag=f"out{oi}{j}",
                                                 name=f"out{oi}{j}")
                                     for j in range(2)] for oi in range(4)]
                            first_pass = (ocp == 0 and tgp == 0)
                            if first_pass:
                                # Wo strips stream between the og rhs loads in
                                # consumption order (head-major), so neither
                                # starves the other on the sync ring
                                for k in range(6):
                                    load_wo(eb_order[k])
                            for ei, eb in enumerate(eb_order):
                                if first_pass and ei + 6 < NC:
                                    load_wo(eb_order[ei + 6])
                                r, hl = eb // HPC, eb % HPC
                                rhs_t = work3.tile([128, 1024], BF, tag="rhs",
                                                   bufs=12)
                                if hl == HPC - 1:
                                    src = og_out7[tgp][r * 128:(r + 1) * 128, :]
                                else:
                                    src = og_out[hl][r * 128:(r + 1) * 128,
                                                     tgp * 1024:(tgp + 1) * 1024]
                                nc.sync.dma_start(out=rhs_t[:], in_=src)
                                for oi in range(4):
                                    oc = ocp * 4 + oi
                                    w = woT_sb[:, eb * EQ + oc * 128:
                                               eb * EQ + (oc + 1) * 128]
                                    for j in range(2):
                                        nc.tensor.matmul(
                                            accs[oi][j][:],
                                            w,
                                            rhs_t[:, j * 512:(j + 1) * 512],
                                            start=(ei == 0),
                                            stop=(ei == NC - 1),
                                        )
                            for oi in range(4):
                                oc = ocp * 4 + oi
                                for j in range(2):
                                    t0 = tgp * 1024 + j * 512
                                    ev = work3.tile([128, 512], F32, tag="ev",
                                                    bufs=4)
                                    # alternate drain engines: halves the
                                    # serial PSUM evacuation at pass ends
                                    if (oi + j) % 2 == 0:
                                        nc.vector.tensor_copy(ev[:],
                                                              accs[oi][j][:])
                                    else:
                                        nc.scalar.copy(ev[:], accs[oi][j][:])
                                    nc.sync.dma_start(
                                        out=out_part[oc * 128:(oc + 1) * 128,
                                                     t0:t0 + 512],
                                        in_=ev[:],
                                    )
    nc.finalize()
    return nc


_NC_CACHE = None


def _get_nc():
    global _NC_CACHE
    if _NC_CACHE is None:
        _NC_CACHE = build()
    return _NC_CACHE


def kernel(x, Wq, Wkv, Wo):
    x = np.asarray(x, dtype=np.float32)
    Wq = np.asarray(Wq, dtype=np.float32)
    Wkv = np.asarray(Wkv, dtype=np.float32)
    Wo = np.asarray(Wo, dtype=np.float32)

    try:
        import ml_dtypes

        bf16 = ml_dtypes.bfloat16
    except ImportError:  # pragma: no cover
        import jax.numpy as jnp

        bf16 = jnp.bfloat16

    xT_b = [np.ascontiguousarray(x[b].T).astype(bf16) for b in range(B)]

    kl = np.arange(128)[:, None]
    ql = np.arange(512)[None, :]
    masks = []
    for pos in range(2):
        cols = []
        for j in range(2):
            jd = 2 * pos + j
            cols.append((kl <= ql - 128 * jd).astype(np.float32))
        masks.append(np.concatenate(cols, axis=1).astype(bf16))
    maskA_np, maskB_np = masks

    ident = np.eye(128, dtype=np.float32).astype(bf16)
    ones = np.ones((128, 1), dtype=np.float32).astype(bf16)

    in_maps = []
    for c in range(NCORES):
        b, j = c // 4, c % 4
        wq_l = Wq[EQ * j:EQ * (j + 1), :]  # [1024, 4096]
        wk_l = Wkv[EKV * j:EKV * (j + 1), :]  # [256, 4096]
        wv_l = Wkv[N_KV * HD + EKV * j:N_KV * HD + EKV * (j + 1), :]
        wkv = np.concatenate([wk_l, wv_l], axis=0)  # [512, 4096]
        wkvT_l = np.ascontiguousarray(wkv.T).astype(bf16)  # [4096, 512]
        # Q weights in SBUF layout: row h*128+p, col cb*128+q =
        # WqT[cb*128+p, h*128+q]
        wqT_l = wq_l.T.reshape(NC, 128, HPC, 128)  # (cb, p, h, q)
        wqsb_l = np.ascontiguousarray(
            wqT_l.transpose(2, 1, 0, 3).reshape(HPC * 128, NC * 128)
        ).astype(bf16)
        woT_l = np.ascontiguousarray(Wo[EQ * j:EQ * (j + 1), :].T).astype(bf16)
        in_maps.append(
            {
                "xT": xT_b[b],
                "wkvT": wkvT_l,
                "wqsb": wqsb_l,
                "woT": woT_l,
                "maskA": maskA_np,
                "maskB": maskB_np,
                "ident": ident,
                "ones_in": ones,
            }
        )

    nc = _get_nc()
    res = run_bass_kernel_spmd(nc, in_maps, core_ids=list(range(NCORES)))

    out = np.empty((B, T, DIM), dtype=np.float32)
    for b in range(B):
        outT = np.concatenate(
            [res.results[b * 4 + j]["out_part"] for j in range(4)], axis=0
        )  # [4096, 2048]
        out[b] = outT.T
    return out
